# revision 1
# baseline (speedup 1.0000x reference)
# Trainium2 Bass kernel: nn_DecoderAttentionLayer (sliding-window decoder layer)
# Sequence-parallel over 8 NeuronCores: core = (n, quarter); each core processes
# 1024 tokens (+128-token halo for the previous key/value chunk).
#
# Per-core pipeline (all-transposed-free design):
#   x -> bn_stats rms -> qkv matmul in [t, j] (raw x; rms folded into scales)
#   q,k: square/reduce stats, scale (inv * rsqrt(mean+eps) [*1/8 for q]) + rotary
#        -> PE transpose to [d, t] (bf16)
#   scores [tq, tk] = qT.T @ kT ; exp (ACT) ; mask+denominator (DVE) ; normalize
#   probs -> PE transpose -> attn_out^T accumulated in PSUM -> o_proj + residual
import sys
import numpy as np
import ml_dtypes

sys.path.insert(0, "/opt/trn_rl_repo")

import bass_rust
import concourse.bass as bass
import concourse.tile as tile
from concourse import mybir
from concourse.bass_utils import run_bass_kernel_spmd
from concourse.vector_clock import ScopedClock

F32 = mybir.dt.float32
BF16 = mybir.dt.bfloat16
AF = mybir.ActivationFunctionType
ALU = mybir.AluOpType
BF = ml_dtypes.bfloat16

N, T, D = 2, 4096, 1024
HD, NH, W = 64, 16, 128
EPS = 1.1920929e-07
TLOC = 1152          # 128 halo + 1024 own tokens
NCH = 9              # x chunks per core (chunk 0 = halo)
NPAIR = 8            # head pairs


# ---------------------------------------------------------------------------
# Compiler workarounds: walrus in this container accepts at most ONE sem wait
# per instruction on most structs. Split excess waits onto NoOps.
# ---------------------------------------------------------------------------
def _split_excess_waits(nc):
    cnt = 0
    for f in nc.m.functions:
        for b in f.blocks:
            changed = False
            new_insts = []
            for inst in b.instructions:
                si = inst.sync_info
                waits = list(si.on_wait) if (si is not None and si.on_wait) else []
                if len(waits) > 1:
                    si.on_wait = waits[:1]
                    for w in waits[1:]:
                        cnt += 1
                        nop = bass_rust.InstNoOp(
                            name=f"I-waitfix-{cnt}", engine=inst.engine)
                        nop.sync_info = mybir.SyncInfo(on_wait=[w], on_update=[])
                        new_insts.append(nop)
                    changed = True
                new_insts.append(inst)
            if changed:
                b.instructions = new_insts
    return cnt


def _patched_drain_and_barrier(self, tick_clock, wait_clock):
    drain_inst = self.nc.sync.drain()
    wait_clock.add_sem_waits(
        drain_inst.ins, ScopedClock({None: tick_clock.global_clock}))
    si = drain_inst.ins.sync_info
    if si is not None and si.on_wait and len(si.on_wait) > 1:
        waits = list(si.on_wait)
        si.on_wait = waits[:1]
        for w in waits[1:]:
            extra = self.nc.sync.drain()
            esi = extra.ins.sync_info
            if esi is None:
                extra.ins.sync_info = mybir.SyncInfo(on_wait=[w], on_update=[])
            else:
                esi.on_wait = [w]
    self.nc.all_engine_barrier()
    assert self.sems is not None
    popped = self.nc._tile_sem_poison_stack.pop()
    assert popped is self._sem_poison
    self.nc.clear_and_free_semaphores(list(self.sems.allocated().values()))
    self.nc.all_engine_barrier()


tile.TileContext._drain_and_barrier = _patched_drain_and_barrier


def _ap(t, offset, dims):
    return bass.AP(tensor=t.tensor, offset=t.offset + offset, ap=[t.ap[0]] + dims)


def build_program(waitfix=True, phases=3, subB=4):
    nc = bass.Bass()

    x_nat = nc.dram_tensor("x_nat", [TLOC, D], F32, kind="ExternalInput")
    xT = nc.dram_tensor("xT", [D, TLOC], BF16, kind="ExternalInput")
    wT = nc.dram_tensor("wT", [D, 3 * D], BF16, kind="ExternalInput")
    owsT = nc.dram_tensor("owsT", [D, D], BF16, kind="ExternalInput")
    rot = nc.dram_tensor("rot", [TLOC, 1024], BF16, kind="ExternalInput")
    maskF = nc.dram_tensor("maskF", [W, 2 * W], BF16, kind="ExternalInput")
    maskR = nc.dram_tensor("maskR", [W, 2 * W], BF16, kind="ExternalInput")
    eye = nc.dram_tensor("eye", [128, 128], BF16, kind="ExternalInput")
    y = nc.dram_tensor("y", [1024, D], F32, kind="ExternalOutput")

    with tile.TileContext(nc) as tc:
        with tc.tile_pool(name="persist", bufs=1) as P, \
             tc.tile_pool(name="xpool", bufs=2) as XP, \
             tc.tile_pool(name="cspool", bufs=2) as CS, \
             tc.tile_pool(name="qkpool", bufs=2) as QK, \
             tc.tile_pool(name="small", bufs=4) as SM, \
             tc.tile_pool(name="probs", bufs=5) as PR, \
             tc.tile_pool(name="ypool", bufs=2) as YP, \
             tc.tile_pool(name="ps_big", bufs=2, space="PSUM") as PSB, \
             tc.tile_pool(name="ps_tr", bufs=2, space="PSUM") as PST, \
             tc.tile_pool(name="ps_u", bufs=2, space="PSUM") as PSU:

            # ---------------- persistent loads ----------------
            wT_k = []
            for kt in range(8):
                t = P.tile([128, 3 * D], BF16, tag=f"wT{kt}")
                nc.sync.dma_start(out=t, in_=wT[kt * 128:(kt + 1) * 128, :])
                wT_k.append(t)
            ow_k = []
            for kt in range(8):
                t = P.tile([128, D], BF16, tag=f"ow{kt}")
                nc.sync.dma_start(out=t, in_=owsT[kt * 128:(kt + 1) * 128, :])
                ow_k.append(t)
            xT_k = []
            for kt in range(8):
                t = P.tile([128, TLOC], BF16, tag=f"xT{kt}")
                nc.sync.dma_start(out=t, in_=xT[kt * 128:(kt + 1) * 128, :])
                xT_k.append(t)
            eye_t = P.tile([128, 128], BF16, tag="eye")
            nc.sync.dma_start(out=eye_t, in_=eye[:, :])
            mF = P.tile([W, 2 * W], BF16, tag="mF")
            nc.sync.dma_start(out=mF, in_=maskF[:, :])
            mR = P.tile([W, 2 * W], BF16, tag="mR")
            nc.sync.dma_start(out=mR, in_=maskR[:, :])
            eps_t = P.tile([128, 1], F32, tag="eps")
            nc.vector.memset(eps_t, EPS)

            # persistent big activation stores (bf16)
            qT_all = P.tile([128, NPAIR * TLOC], BF16, tag="qT_all")
            kT_all = P.tile([128, NPAIR * TLOC], BF16, tag="kT_all")
            attn_T = P.tile([128, NPAIR * 1024], BF16, tag="attn_T")
            inv_all = P.tile([128, NCH], F32, tag="inv_all")
            sq_all = P.tile([128, NCH * NH], F32, tag="sq_all")
            v_sb = []

            # ================= phase A: per chunk qkv + q/k prep ============
            for c in range(NCH):
                xt = XP.tile([128, D], F32, tag="x")
                nc.sync.dma_start(out=xt, in_=x_nat[c * 128:(c + 1) * 128, :])
                cs = CS.tile([128, 1024], BF16, tag="cs")
                nc.sync.dma_start(out=cs, in_=rot[c * 128:(c + 1) * 128, :])

                # ---- x rms stats: inv = 1/sqrt(mean(x^2) + eps)
                bstats = SM.tile([128, 2, 6], F32, tag="bstats")
                for g in range(2):
                    nc.vector.bn_stats(out=bstats[:, g, :],
                                       in_=xt[:, g * 512:(g + 1) * 512])
                mv = SM.tile([128, 2], F32, tag="mv")
                nc.vector.bn_aggr(out=mv, in_=bstats)
                msq = SM.tile([128, 1], F32, tag="msq")
                nc.vector.tensor_mul(msq, mv[:, 0:1], mv[:, 0:1])
                nc.vector.tensor_add(msq, msq, mv[:, 1:2])
                rsq = SM.tile([128, 1], F32, tag="rsq")
                nc.scalar.activation(out=rsq, in_=msq, func=AF.Sqrt, bias=eps_t)
                inv = inv_all[:, c:c + 1]
                nc.vector.reciprocal(out=inv, in_=rsq)
                inv2 = SM.tile([128, 1], F32, tag="inv2")
                nc.vector.tensor_mul(inv2, inv, inv)

                # ---- qkv matmuls -> psum [t, j] (q, k, v separately)
                def qkv_mm(jlo):
                    ps = PSB.tile([128, 1024], F32, tag="qkv_ps")
                    for half in range(2):
                        for kt in range(8):
                            nc.tensor.matmul(
                                ps[:, half * 512:(half + 1) * 512],
                                xT_k[kt][:, c * 128:(c + 1) * 128],
                                wT_k[kt][:, jlo + half * 512: jlo + (half + 1) * 512],
                                start=(kt == 0), stop=(kt == 7))
                    return ps

                # ---- V: evacuate with inv scale folded (ACT copy)
                v_ps = qkv_mm(2048)
                vt = P.tile([128, 1024], BF16, tag=f"v{c}")
                for half in range(2):
                    nc.scalar.activation(
                        out=vt[:, half * 512:(half + 1) * 512],
                        in_=v_ps[:, half * 512:(half + 1) * 512],
                        func=AF.Copy, scale=inv)
                v_sb.append(vt)

                # ---- Q and K
                for which, jlo in (("q", 0), ("k", 1024)):
                    ps = qkv_mm(jlo)
                    sq = QK.tile([128, 1024], BF16, tag="sq")
                    for half in range(2):
                        nc.scalar.activation(
                            out=sq[:, half * 512:(half + 1) * 512],
                            in_=ps[:, half * 512:(half + 1) * 512], func=AF.Square)
                    raw = QK.tile([128, 1024], BF16, tag=f"{which}raw")
                    nc.any.tensor_copy(raw, ps[:, :])
                    ssq = SM.tile([128, NH], F32, tag="ssq")
                    nc.vector.tensor_reduce(
                        out=ssq, in_=sq.rearrange("p (h d) -> p h d", h=NH),
                        axis=mybir.AxisListType.X, op=ALU.add)
                    # m_true = ssq * inv^2 / 64 ; r = 1/sqrt(m_true + eps)
                    mt = SM.tile([128, NH], F32, tag="mt")
                    nc.vector.tensor_scalar(
                        out=mt, in0=ssq, scalar1=inv2, scalar2=1.0 / 64.0,
                        op0=ALU.mult, op1=ALU.mult)
                    rs = SM.tile([128, NH], F32, tag="rs")
                    nc.scalar.activation(out=rs, in_=mt, func=AF.Sqrt, bias=eps_t)
                    rr = SM.tile([128, NH], F32, tag="rr")
                    nc.vector.reciprocal(out=rr, in_=rs)
                    if which == "q":
                        # q scale is applied later, as the ACT-exp per-
                        # partition scale operand in phase B
                        scl = sq_all[:, c * NH:(c + 1) * NH]
                        nc.vector.tensor_scalar(
                            out=scl, in0=rr, scalar1=inv, scalar2=0.125,
                            op0=ALU.mult, op1=ALU.mult)
                    else:
                        scl = SM.tile([128, NH], F32, tag="scl")
                        nc.vector.tensor_scalar_mul(
                            out=scl, in0=rr, scalar1=inv)
                        # scale k tile: raw *= scl (broadcast over d)
                        nc.vector.tensor_mul(
                            raw.rearrange("p (h d) -> p h d", h=NH),
                            raw.rearrange("p (h d) -> p h d", h=NH),
                            _ap(scl, 0, [[1, NH], [0, HD]]))
                    # rotary on active 16-col blocks (d in 0:16 and 32:48);
                    # cs cols 0:512 = cos (expanded), 512:1024 = +/-sin
                    qs = QK.tile([128, NH, 2, 16], BF16, tag="qs")
                    nc.vector.tensor_copy(
                        qs, _ap(raw, 32, [[64, NH], [-32, 2], [1, 16]]))
                    t1 = QK.tile([128, NH, 2, 16], BF16, tag="t1")
                    nc.vector.tensor_mul(
                        t1, qs, _ap(cs, 512, [[32, NH], [16, 2], [1, 16]]))
                    act = _ap(raw, 0, [[64, NH], [32, 2], [1, 16]])
                    nc.gpsimd.tensor_mul(
                        act, act, _ap(cs, 0, [[32, NH], [16, 2], [1, 16]]))
                    nc.gpsimd.tensor_add(act, act, t1)
                    # transpose 8 head-pairs -> {q,k}T_all[:, p*TLOC + c*128 ...]
                    dst = qT_all if which == "q" else kT_all
                    for grp in range(2):
                        tp = PST.tile([128, 512], BF16, tag="tp")
                        for i in range(4):
                            p = grp * 4 + i
                            nc.tensor.transpose(
                                tp[:, i * 128:(i + 1) * 128],
                                raw[:, p * 128:(p + 1) * 128], eye_t)
                        nc.any.tensor_copy(
                            _ap(dst, (grp * 4) * TLOC + c * 128,
                                [[TLOC, 4], [1, 128]]), tp)

            if phases >= 2:
              # =============== phase B: attention per own chunk =============
              for c in range(1, NCH):
                  mask = mF if c == 1 else mR
                  for grp in range(2):
                      pT_grp = []
                      for i in range(4):
                          p = grp * 4 + i
                          # two concurrent row-group matmuls must land in
                          # different PSUM banks (same-bank dual drain hangs)
                          s_ps = PSB.tile([128, 1024], F32, tag="qkv_ps")
                          for hh in range(2):
                              off = p * TLOC
                              nc.tensor.matmul(
                                  s_ps[:, hh * 512: hh * 512 + 256],
                                  qT_all[hh * 64:(hh + 1) * 64,
                                         off + c * 128: off + (c + 1) * 128],
                                  kT_all[hh * 64:(hh + 1) * 64,
                                         off + (c - 1) * 128: off + (c + 1) * 128],
                                  start=True, stop=True)
                          e_sb = PR.tile([128, 512], BF16, tag="e_sb")
                          for hh in range(2):
                              h = 2 * p + hh
                              nc.scalar.activation(
                                  out=e_sb[:, hh * 256:(hh + 1) * 256],
                                  in_=s_ps[:, hh * 512: hh * 512 + 256],
                                  func=AF.Exp,
                                  scale=sq_all[:, c * NH + h: c * NH + h + 1])
                          # multiplicative 0/1 mask on the idle gpsimd engine
                          nc.gpsimd.tensor_mul(
                              e_sb.rearrange("p (h k) -> p h k", h=2),
                              e_sb.rearrange("p (h k) -> p h k", h=2),
                              _ap(mask, 0, [[0, 2], [1, 256]]))
                          den = SM.tile([128, 2], F32, tag="den")
                          nc.vector.tensor_reduce(
                              out=den,
                              in_=e_sb.rearrange("p (h k) -> p h k", h=2),
                              axis=mybir.AxisListType.X, op=ALU.add)
                          invd = SM.tile([128, 2], F32, tag="invd")
                          nc.vector.reciprocal(out=invd, in_=den)
                          for hh in range(2):
                              nc.vector.tensor_scalar_mul(
                                  out=e_sb[:, hh * 256:(hh + 1) * 256],
                                  in0=e_sb[:, hh * 256:(hh + 1) * 256],
                                  scalar1=invd[:, hh:hh + 1])
                          if subB <= 1:
                              pT_grp.append(e_sb)
                              continue
                          ptp = PST.tile([128, 512], BF16, tag="tp")
                          for i4 in range(4):
                              nc.tensor.transpose(
                                  ptp[:, i4 * 128:(i4 + 1) * 128],
                                  e_sb[:, i4 * 128:(i4 + 1) * 128], eye_t)
                          pT = PR.tile([128, 512], BF16, tag="pT")
                          nc.any.tensor_copy(pT, ptp)
                          pT_grp.append(pT)
                      if subB <= 2:
                          continue
                      # attn-out: U[128(2h x 64d), 128tq] per pair, 4 pairs/psum
                      u_ps = PSU.tile([128, 512], F32, tag="u_ps")
                      for i in range(4):
                          p = grp * 4 + i
                          pT = pT_grp[i]
                          for hh in range(2):
                              h = 2 * p + hh
                              for kc in range(2):
                                  nc.tensor.matmul(
                                      u_ps[hh * 64:(hh + 1) * 64,
                                           i * 128:(i + 1) * 128],
                                      v_sb[c - 1 + kc][:, h * 64:(h + 1) * 64],
                                      pT[:, (2 * hh + kc) * 128:
                                            (2 * hh + kc + 1) * 128],
                                      start=(kc == 0), stop=(kc == 1),
                                      tile_position=(0, hh * 64))
                      nc.any.tensor_copy(
                          _ap(attn_T, (grp * 4) * 1024 + (c - 1) * 128,
                              [[1024, 4], [1, 128]]), u_ps)

            if phases >= 3:
              # =============== phase C: o_proj + residual ===================
              for c in range(1, NCH):
                  o_ps = PSB.tile([128, 1024], F32, tag="qkv_ps")
                  for half in range(2):
                      for kt in range(8):
                          nc.tensor.matmul(
                              o_ps[:, half * 512:(half + 1) * 512],
                              attn_T[:, kt * 1024 + (c - 1) * 128:
                                     kt * 1024 + c * 128],
                              ow_k[kt][:, half * 512:(half + 1) * 512],
                              start=(kt == 0), stop=(kt == 7))
                  xr = XP.tile([128, D], F32, tag="xr")
                  nc.sync.dma_start(out=xr, in_=x_nat[c * 128:(c + 1) * 128, :])
                  for half in range(2):
                      yt = YP.tile([128, 512], F32, tag="y")
                      nc.vector.tensor_add(
                          yt, o_ps[:, half * 512:(half + 1) * 512],
                          xr[:, half * 512:(half + 1) * 512])
                      nc.sync.dma_start(
                          out=y[(c - 1) * 128:c * 128,
                                half * 512:(half + 1) * 512], in_=yt)

            if phases < 3:
                dbg = YP.tile([128, 512], F32, tag="y")
                dsrc = attn_T if (phases >= 2 and subB >= 3) else qT_all
                nc.vector.tensor_copy(dbg, dsrc[:, 0:512])
                nc.sync.dma_start(out=y[0:128, 0:512], in_=dbg)

    if waitfix:
        _split_excess_waits(nc)
    return nc


_PROGRAM = None


def _get_program():
    global _PROGRAM
    if _PROGRAM is None:
        _PROGRAM = build_program()
    return _PROGRAM


def _host_inputs(input_NTD, qkv_weight, o_weight, o_scale):
    x = np.asarray(input_NTD, dtype=np.float32)
    wq = np.asarray(qkv_weight, dtype=np.float32).reshape(3 * D, D)
    wT = np.ascontiguousarray(wq.T).astype(BF)
    ows = np.asarray(o_weight, dtype=np.float32) * \
        np.asarray(o_scale, dtype=np.float32)[:, None]
    owsT = np.ascontiguousarray(ows.T).astype(BF)
    eye = np.eye(128, dtype=np.float32).astype(BF)

    j = np.arange(W)[:, None]
    m = np.arange(2 * W)[None, :]
    base = (m > j) & (m <= W + j)
    maskR = (-300.0 * (1.0 - base)).astype(np.float32).astype(BF)
    maskF0 = (-300.0 * (1.0 - (base & (m >= W)))).astype(np.float32).astype(BF)

    freqs = (1.0 / 10000.0) ** np.linspace(0.0, 1.0, 16).astype(np.float32)

    in_maps = []
    for core in range(8):
        n, qq = divmod(core, 4)
        lo = qq * 1024 - 128
        if qq == 0:
            xs = np.concatenate(
                [np.zeros((128, D), np.float32), x[n, 0:1024]], axis=0)
        else:
            xs = x[n, lo:lo + 1024 + 128]
        xs = np.ascontiguousarray(xs)
        pos = np.maximum(np.arange(lo, lo + TLOC), 0).astype(np.float32)
        theta = pos[:, None] * freqs[None, :]
        cos16, sin16 = np.cos(theta), np.sin(theta)
        # expanded per-head tables: per head h, 32 cols [cos16|cos16] and
        # [sin16|-sin16]; heads identical -> tile 16x
        c32 = np.concatenate([cos16, cos16], axis=1)
        s32 = np.concatenate([sin16, -sin16], axis=1)
        rot = np.concatenate(
            [np.tile(c32, (1, NH)), np.tile(s32, (1, NH))], axis=1).astype(BF)
        in_maps.append(dict(
            x_nat=xs,
            xT=np.ascontiguousarray(xs.T).astype(BF),
            wT=wT, owsT=owsT, rot=np.ascontiguousarray(rot),
            maskF=(maskF0 if qq == 0 else maskR), maskR=maskR, eye=eye))
    return in_maps


def kernel(input_NTD, qkv_weight, o_weight, o_scale, _trace=False):
    nc = _get_program()
    in_maps = _host_inputs(input_NTD, qkv_weight, o_weight, o_scale)
    res = run_bass_kernel_spmd(nc, in_maps, core_ids=list(range(8)),
                               trace=_trace)
    kernel.last_results = res
    out = np.empty((N, T, D), dtype=np.float32)
    for core in range(8):
        n, qq = divmod(core, 4)
        out[n, qq * 1024:(qq + 1) * 1024] = res.results[core]["y"]
    return out

